# revision 16
# baseline (speedup 1.0000x reference)
"""HDTimeCrystalBlock kernel for 8 Trainium2 NeuronCores.

Math: out = ((x @ W_in) * mod[None]) @ W_out, where
  mod[l,h] = sum_m coupled[m] * cos(omega*(m+1)*t[l] + E[m,h])
With cos(a+b) = cos(a)cos(b) - sin(a)sin(b), mod is a K=32 matmul:
  mod[h,l] = sum_r ab[r,h] * cs[r,l]
  ab rows 0:16  = coupled[m]*cos(E[m,h]),  rows 16:32 = -coupled[m]*sin(E[m,h])
  cs rows 0:16  = cos(omega*(m+1)*t[l]),   rows 16:32 = sin(...)
ab/cs are tiny and precomputed on the HOST (the old on-device prep chain
kept the PE HAM-throttled at 1.2 GHz for the first ~35us).  Both are
shipped with the 32 rows duplicated at partitions 32:64 so consecutive
j-tiles' mod matmuls run CONCURRENTLY in different 32-row strips of the
PE array (row tiling): a pair of K=32 matmuls costs ~one N=512 slot.

Sharding: data-parallel over the 8192 tokens (B*L), 1024 per core; weights
replicated. Activations stay transposed ([feature, token]); all matmul
operands bf16 (1 col/cycle @ 2.4 GHz). Output stored bf16.

Startup discipline:
  - PE warm-up matmuls on a memset scratch tile (no DMA dependency): PE
    busy from ~7us, HAM un-throttles by ~10.5us when real data lands.
  - DMA queue heads are exactly the first-needed tiles (win j0-1 on sync,
    xts0 on scalar, ab+cs on gpsimd); the remaining ~8.5MB of weight
    streams are gated behind a 1-element copy that depends on xts0, so
    they cannot steal HBM bandwidth from the critical path.
Steady-loop discipline:
  - The main loop runs in j-PAIRS; the mm2 stage is software-pipelined one
    pair (two j) behind, so the pb->msb->hm chain has ~4.5us of slack.
  - mm2 writes 8 half-bank PSUM tiles ([128,256] x 2 halves x 4 d-tiles =
    4 banks, pool bufs=8) so consecutive q's outputs double-buffer and the
    yo output copies (interleaved on Act/DVE during the next q) are never
    on the critical path.
"""
import math

import numpy as np

B, L, D, HD, M = 4, 2048, 512, 4096, 16
NCORES = 8
T = (B * L) // NCORES          # tokens per core
QCH = 512                      # l-chunk (PSUM bank width in fp32)
HCH = QCH // 2                 # mm2 half-tile width
NQ = T // QCH
NJ = HD // 128                 # h-tiles
NK = D // 128                  # d-tiles
K2 = 2 * M                     # mod-matmul contraction (32)
NWARM = 11                     # scratch warm-up matmuls (N=512)

# j-tile ranges per DMA chunk for w_in / w_out ([lo, hi) in j-tiles)
WIN_PARTS = [(0, 2), (2, 4), (4, 8), (8, 12), (12, 16), (16, 20),
             (20, 24), (24, 28), (28, 32)]
WOUT_PARTS = [(0, 4), (4, 8), (8, 12), (12, 16), (16, 20),
              (20, 24), (24, 28), (28, 32)]

_cache = {}


def _build():
    from concourse import bacc, bass, mybir, tile

    F32 = mybir.dt.float32
    BF16 = mybir.dt.bfloat16
    PSUM = bass.MemorySpace.PSUM

    nc = bacc.Bacc("TRN2", target_bir_lowering=False, debug=False)

    xT_d = nc.dram_tensor("xT", [D, T], BF16, kind="ExternalInput")
    w_in_d = nc.dram_tensor("w_in", [D, HD], BF16, kind="ExternalInput")
    w_out_d = nc.dram_tensor("w_out", [HD, D], BF16, kind="ExternalInput")
    cs_d = nc.dram_tensor("cs", [2 * K2, T], BF16, kind="ExternalInput")
    ab_d = nc.dram_tensor("ab", [2 * K2, HD], BF16, kind="ExternalInput")
    yT_d = nc.dram_tensor("yT", [D, T], BF16, kind="ExternalOutput")
    gate_d = nc.dram_tensor("gatescr", [1, NK], BF16, kind="Internal")

    with tile.TileContext(nc) as tc:
        with (
            tc.tile_pool(name="win", bufs=1) as winp,
            tc.tile_pool(name="wout", bufs=1) as woutp,
            tc.tile_pool(name="xts", bufs=1) as xtp,
            tc.tile_pool(name="small", bufs=1) as smallp,
            tc.tile_pool(name="hm", bufs=4) as hmp,
            tc.tile_pool(name="mods", bufs=4) as modsp,
            tc.tile_pool(name="yo", bufs=4) as yop,
            tc.tile_pool(name="pa", bufs=2, space=PSUM) as pap,
            tc.tile_pool(name="pb", bufs=2, space=PSUM) as pbp,
            tc.tile_pool(name="py", bufs=4, space=PSUM) as pyp,
        ):
            # ---- scratch warm-up tile, memset on the otherwise-idle DVE ----
            wmt = smallp.tile([128, QCH], BF16, tag="wmt")
            nc.vector.memset(wmt[:], 0.5)

            # ---- DMA schedule ----
            w_in_r = w_in_d.ap().rearrange("(k p) h -> p k h", p=128)
            w_out_r = w_out_d.ap().rearrange("(j p) i -> p j i", p=128)
            xT_r = xT_d.ap().rearrange("(k p) (q t) -> q p k t", p=128, q=NQ)

            win_c = [None] * len(WIN_PARTS)
            wout_g = [None] * len(WOUT_PARTS)
            xts_q = [None] * NQ

            def load_win(i):
                a, b = WIN_PARTS[i]
                t_ = winp.tile([128, NK, 128 * (b - a)], BF16,
                               name=f"win{i}", tag=f"win{i}")
                nc.sync.dma_start(t_[:], w_in_r[:, :, 128 * a : 128 * b])
                win_c[i] = t_

            def load_wout(i):
                a, b = WOUT_PARTS[i]
                tw = woutp.tile([128, b - a, D], BF16,
                                name=f"wout{i}", tag=f"wout{i}")
                nc.gpsimd.dma_start(tw[:], w_out_r[:, a:b, :])
                wout_g[i] = tw

            def load_xts(q, eng):
                tx = xtp.tile([128, NK, T // NQ], BF16, name=f"xts{q}", tag=f"xts{q}")
                eng.dma_start(tx[:], xT_r[q])
                xts_q[q] = tx

            ab = smallp.tile([2 * K2, HD], BF16, tag="ab")
            cs = smallp.tile([2 * K2, T], BF16, tag="cs")
            gate2 = smallp.tile([1, NK], BF16, tag="gate2")
            gate3 = smallp.tile([1, NK], BF16, tag="gate3")

            # critical set, balanced across the 3 DMA queues (~115 GB/s each):
            #   scalar: xts0 k0-1 (256K) + cs (128K);  sync: xts0 k2-3 (256K)
            #   + win j0-1 (256K);  gpsimd: ab (512K)
            tx0 = xtp.tile([128, NK, T // NQ], BF16, name="xts0", tag="xts0")
            xts_q[0] = tx0
            nc.scalar.dma_start(tx0[:, 0:2, :], xT_r[0][:, 0:2, :])
            nc.sync.dma_start(tx0[:, 2:4, :], xT_r[0][:, 2:4, :])
            load_win(0)                   # sync: j0-1
            nc.scalar.dma_start(cs[:], cs_d[:])
            nc.gpsimd.dma_start(ab[:], ab_d[:])
            load_win(1)                   # sync: j2-3
            # per-queue gates: nothing below moves until xts0 has landed
            nc.sync.dma_start(gate_d[0:1, :], tx0[0:1, :, 0:1])       # sync gate
            nc.scalar.copy(gate2[:], tx0[0:1, :, 0:1])                # scalar gate
            nc.gpsimd.tensor_copy(gate3[:], tx0[0:1, :, 0:1])         # gpsimd gate
            load_wout(0)
            for i in range(2, len(WIN_PARTS)):
                load_win(i)
            for i in range(1, len(WOUT_PARTS)):
                load_wout(i)
            load_xts(1, nc.scalar)

            def win_slice(j, k):
                for i, (a, b) in enumerate(WIN_PARTS):
                    if a <= j < b:
                        return win_c[i][:, k, 128 * (j - a) : 128 * (j - a + 1)]
                raise AssertionError

            def wout_slice(j, j2):
                for i, (a, b) in enumerate(WOUT_PARTS):
                    if a <= j < b:
                        return wout_g[i][:, j - a, 128 * j2 : 128 * (j2 + 1)]
                raise AssertionError

            # ---- PE warm-up on scratch (HAM to K=8/8 by ~10.5us) ----
            for w in range(NWARM):
                pw = pap.tile([128, QCH], F32, name=f"warm{w}", tag="pa")
                nc.tensor.matmul(pw[:], wmt[:, 0:128], wmt[:], start=True, stop=True)

            def emit_yo_batch(pq, ppys):
                # 2 copies on Act, 2 on DVE, stores on the idle sync queue
                yos = []
                for j2 in range(NK):
                    yo = yop.tile([128, QCH], BF16, name=f"yo{pq}_{j2}", tag="yo")
                    if j2 % 2 == 0:
                        nc.scalar.copy(yo[:], ppys[j2][:])
                    else:
                        nc.vector.tensor_copy(yo[:], ppys[j2][:])
                    yos.append(yo)
                for j2 in range(NK):
                    nc.sync.dma_start(
                        yT_d[128 * j2 : 128 * (j2 + 1),
                             pq * QCH : (pq + 1) * QCH],
                        yos[j2][:],
                    )

            # ---- fused main loop: j-pairs, mm2 pipelined one pair behind ----
            prev_q = None  # (q, pys) drained at pair 1 of the next q
            for q in range(NQ):
                lo, hi = q * QCH, (q + 1) * QCH
                pys = [pyp.tile([128, QCH], F32, name=f"py{q}_{j2}", tag="py")
                       for j2 in range(NK)]
                pend = None
                for p in range(NJ // 2):
                    j0, j1 = 2 * p, 2 * p + 1
                    pa0 = pap.tile([128, QCH], F32, tag="pa")
                    for k in range(NK):
                        nc.tensor.matmul(pa0[:], win_slice(j0, k),
                                         xts_q[q][:, k, :],
                                         start=(k == 0), stop=(k == NK - 1))
                    pb0 = pbp.tile([128, QCH], F32, tag="pb")
                    nc.tensor.matmul(pb0[:], ab[0:K2, 128 * j0 : 128 * (j0 + 1)],
                                     cs[0:K2, lo:hi], start=True, stop=True)
                    pb1 = pbp.tile([128, QCH], F32, tag="pb")
                    nc.tensor.matmul(pb1[:],
                                     ab[K2 : 2 * K2, 128 * j1 : 128 * (j1 + 1)],
                                     cs[K2 : 2 * K2, lo:hi],
                                     start=True, stop=True)
                    msb0 = modsp.tile([128, QCH], F32, tag="mods")
                    nc.scalar.copy(msb0[:], pb0[:])
                    hm0 = hmp.tile([128, QCH], BF16, tag="hm")
                    nc.vector.tensor_mul(hm0[:], pa0[:], msb0[:])
                    pa1 = pap.tile([128, QCH], F32, tag="pa")
                    for k in range(NK):
                        nc.tensor.matmul(pa1[:], win_slice(j1, k),
                                         xts_q[q][:, k, :],
                                         start=(k == 0), stop=(k == NK - 1))
                    msb1 = modsp.tile([128, QCH], F32, tag="mods")
                    nc.scalar.copy(msb1[:], pb1[:])
                    hm1 = hmp.tile([128, QCH], BF16, tag="hm")
                    nc.vector.tensor_mul(hm1[:], pa1[:], msb1[:])
                    # previous q's outputs drain here (after pair 0's msb/mul
                    # so the pa/pb recycle chain is never behind the copies)
                    if p == 1 and prev_q is not None:
                        pq, ppys = prev_q
                        emit_yo_batch(pq, ppys)
                        prev_q = None
                    if pend is not None:
                        for (pj, phm) in pend:
                            for j2 in range(NK):
                                nc.tensor.matmul(
                                    pys[j2][:],
                                    wout_slice(pj, j2),
                                    phm[:],
                                    start=(pj == 0),
                                    stop=(pj == NJ - 1),
                                )
                    pend = [(j0, hm0), (j1, hm1)]
                # flush last pair's mm2; for the final q the yo copies chase
                # the per-j2 stop matmuls so the tail is one copy deep
                last_q = q == NQ - 1
                yos = []
                for (pj, phm) in pend:
                    for j2 in range(NK):
                        nc.tensor.matmul(
                            pys[j2][:],
                            wout_slice(pj, j2),
                            phm[:],
                            start=(pj == 0),
                            stop=(pj == NJ - 1),
                        )
                        if last_q and pj == NJ - 1:
                            yo = yop.tile([128, QCH], BF16,
                                          name=f"yo{q}_{j2}", tag="yo")
                            if j2 % 2 == 0:
                                nc.scalar.copy(yo[:], pys[j2][:])
                            else:
                                nc.vector.tensor_copy(yo[:], pys[j2][:])
                            yos.append((j2, yo))
                for j2, yo in yos:
                    nc.sync.dma_start(
                        yT_d[128 * j2 : 128 * (j2 + 1), lo:hi], yo[:]
                    )
                prev_q = (q, pys)

    nc.finalize()
    return nc


def _get_nc():
    if "nc" not in _cache:
        _cache["nc"] = _build()
    return _cache["nc"]


def _bf16(a):
    import ml_dtypes
    return np.ascontiguousarray(np.asarray(a, dtype=np.float32).astype(ml_dtypes.bfloat16))


def _in_maps(x, input_proj, output_proj, floquet_energies, drive_weights,
             coupling_matrix):
    coupled = coupling_matrix.astype(np.float64) @ drive_weights.astype(np.float64)
    E = floquet_energies.astype(np.float64)
    ab_half = np.concatenate(
        [coupled[:, None] * np.cos(E), -coupled[:, None] * np.sin(E)], axis=0
    )
    ab = _bf16(np.concatenate([ab_half, ab_half], axis=0))
    w_in = _bf16(input_proj)
    w_out = _bf16(output_proj)

    harm = np.arange(1, M + 1, dtype=np.float64)
    maps = []
    for c in range(NCORES):
        b, half = c // 2, c % 2
        t = (half * T + np.arange(T, dtype=np.float64)) / L
        ang = 2.0 * np.pi * harm[:, None] * t[None, :]
        cs_half = np.concatenate([np.cos(ang), np.sin(ang)], axis=0)
        cs = _bf16(np.concatenate([cs_half, cs_half], axis=0))
        xT = _bf16(x[b, half * T : (half + 1) * T, :].T)
        maps.append(
            {
                "xT": xT,
                "w_in": w_in,
                "w_out": w_out,
                "cs": cs,
                "ab": ab,
            }
        )
    return maps


def kernel(x, input_proj, output_proj, floquet_energies, drive_weights,
           coupling_matrix, _trace=False, _trace_kwargs=None):
    from concourse.bass_utils import run_bass_kernel_spmd

    nc = _get_nc()
    maps = _in_maps(x, input_proj, output_proj, floquet_energies,
                    drive_weights, coupling_matrix)
    kw = dict(_trace_kwargs or {})
    res = run_bass_kernel_spmd(nc, maps, list(range(NCORES)), trace=_trace, **kw)
    out = np.empty((B, L, D), dtype=np.float32)
    for c in range(NCORES):
        b, half = c // 2, c % 2
        out[b, half * T : (half + 1) * T, :] = (
            res.results[c]["yT"].astype(np.float32).T
        )
    if _trace:
        return out, res
    return out


# revision 20
# speedup vs baseline: 1.0026x; 1.0026x over previous
"""HDTimeCrystalBlock kernel for 8 Trainium2 NeuronCores.

Math: out = ((x @ W_in) * mod[None]) @ W_out, where
  mod[l,h] = sum_m coupled[m] * cos(omega*(m+1)*t[l] + E[m,h])
With cos(a+b) = cos(a)cos(b) - sin(a)sin(b), mod is a K=32 matmul:
  mod[h,l] = sum_r ab[r,h] * cs[r,l]
  ab rows 0:16  = coupled[m]*cos(E[m,h]),  rows 16:32 = -coupled[m]*sin(E[m,h])
  cs rows 0:16  = cos(omega*(m+1)*t[l]),   rows 16:32 = sin(...)
ab/cs are tiny and precomputed on the HOST (the old on-device prep chain
kept the PE HAM-throttled at 1.2 GHz for the first ~35us).  Both are
shipped with the 32 rows duplicated at partitions 32:64 so consecutive
j-tiles' mod matmuls run CONCURRENTLY in different 32-row strips of the
PE array (row tiling): a pair of K=32 matmuls costs ~one N=512 slot.

Sharding: data-parallel over the 8192 tokens (B*L), 1024 per core; weights
replicated. Activations stay transposed ([feature, token]); all matmul
operands bf16 (1 col/cycle @ 2.4 GHz). Output stored bf16.

Startup discipline:
  - PE warm-up matmuls on a memset scratch tile (no DMA dependency): PE
    busy from ~7us, HAM un-throttles by ~10.5us when real data lands.
  - DMA queue heads are exactly the first-needed tiles (win j0-1 on sync,
    xts0 on scalar, ab+cs on gpsimd); the remaining ~8.5MB of weight
    streams are gated behind a 1-element copy that depends on xts0, so
    they cannot steal HBM bandwidth from the critical path.
Steady-loop discipline:
  - The main loop runs in j-PAIRS; the mm2 stage is software-pipelined one
    pair (two j) behind, so the pb->msb->hm chain has ~4.5us of slack.
  - mm2 writes 8 half-bank PSUM tiles ([128,256] x 2 halves x 4 d-tiles =
    4 banks, pool bufs=8) so consecutive q's outputs double-buffer and the
    yo output copies (interleaved on Act/DVE during the next q) are never
    on the critical path.
"""
import math

import numpy as np

B, L, D, HD, M = 4, 2048, 512, 4096, 16
NCORES = 8
T = (B * L) // NCORES          # tokens per core
QCH = 512                      # l-chunk (PSUM bank width in fp32)
HCH = QCH // 2                 # mm2 half-tile width
NQ = T // QCH
NJ = HD // 128                 # h-tiles
NK = D // 128                  # d-tiles
K2 = 2 * M                     # mod-matmul contraction (32)
NWARM = 9                      # scratch warm-up matmuls (N=512)

# j-tile ranges per DMA chunk for w_in / w_out ([lo, hi) in j-tiles)
WIN_PARTS = [(0, 2), (2, 4), (4, 8), (8, 12), (12, 16), (16, 20),
             (20, 24), (24, 28), (28, 32)]
WOUT_PARTS = [(0, 4), (4, 8), (8, 12), (12, 16), (16, 20),
              (20, 24), (24, 28), (28, 32)]

_cache = {}


def _build():
    from concourse import bacc, bass, mybir, tile

    F32 = mybir.dt.float32
    BF16 = mybir.dt.bfloat16
    PSUM = bass.MemorySpace.PSUM

    nc = bacc.Bacc("TRN2", target_bir_lowering=False, debug=False)

    xT_d = nc.dram_tensor("xT", [D, T], BF16, kind="ExternalInput")
    w_in_d = nc.dram_tensor("w_in", [D, HD], BF16, kind="ExternalInput")
    w_out_d = nc.dram_tensor("w_out", [HD, D], BF16, kind="ExternalInput")
    cs_d = nc.dram_tensor("cs", [2 * K2, T], BF16, kind="ExternalInput")
    ab_d = nc.dram_tensor("ab", [2 * K2, HD], BF16, kind="ExternalInput")
    yT_d = nc.dram_tensor("yT", [D, T], BF16, kind="ExternalOutput")

    with tile.TileContext(nc) as tc:
        with (
            tc.tile_pool(name="win", bufs=1) as winp,
            tc.tile_pool(name="wout", bufs=1) as woutp,
            tc.tile_pool(name="xts", bufs=1) as xtp,
            tc.tile_pool(name="small", bufs=1) as smallp,
            tc.tile_pool(name="hm", bufs=4) as hmp,
            tc.tile_pool(name="mods", bufs=4) as modsp,
            tc.tile_pool(name="yo", bufs=4) as yop,
            tc.tile_pool(name="pa", bufs=2, space=PSUM) as pap,
            tc.tile_pool(name="pb", bufs=2, space=PSUM) as pbp,
            tc.tile_pool(name="py", bufs=4, space=PSUM) as pyp,
        ):
            # ---- scratch warm-up tile, memset on the otherwise-idle DVE ----
            wmt = smallp.tile([128, QCH], BF16, tag="wmt")
            nc.vector.memset(wmt[:], 0.5)

            # ---- DMA schedule ----
            w_in_r = w_in_d.ap().rearrange("(k p) h -> p k h", p=128)
            w_out_r = w_out_d.ap().rearrange("(j p) i -> p j i", p=128)
            xT_r = xT_d.ap().rearrange("(k p) (q t) -> q p k t", p=128, q=NQ)

            win_c = [None] * len(WIN_PARTS)
            wout_g = [None] * len(WOUT_PARTS)
            xts_q = [None] * NQ

            def load_win(i):
                a, b = WIN_PARTS[i]
                t_ = winp.tile([128, NK, 128 * (b - a)], BF16,
                               name=f"win{i}", tag=f"win{i}")
                nc.sync.dma_start(t_[:], w_in_r[:, :, 128 * a : 128 * b])
                win_c[i] = t_

            def load_wout(i):
                a, b = WOUT_PARTS[i]
                tw = woutp.tile([128, b - a, D], BF16,
                                name=f"wout{i}", tag=f"wout{i}")
                nc.gpsimd.dma_start(tw[:], w_out_r[:, a:b, :])
                wout_g[i] = tw

            def load_xts(q, eng):
                tx = xtp.tile([128, NK, T // NQ], BF16, name=f"xts{q}", tag=f"xts{q}")
                eng.dma_start(tx[:], xT_r[q])
                xts_q[q] = tx

            ab = smallp.tile([2 * K2, HD], BF16, tag="ab")
            cs = smallp.tile([2 * K2, T], BF16, tag="cs")
            gate3 = smallp.tile([1, NK], BF16, tag="gate3")

            # critical set, balanced across the 3 DMA queues (~110 GB/s each),
            # relying on per-queue FIFO for the sync/scalar streams:
            #   sync: win j0-1, xts0 k2-3, win j2-3, win rest
            #   scalar: xts0 k0-1, cs
            #   gpsimd: ab, then gate (waits xts0), then wout stream + xts1
            tx0 = xtp.tile([128, NK, T // NQ], BF16, name="xts0", tag="xts0")
            xts_q[0] = tx0
            load_win(0)                   # sync head: j0-1
            nc.scalar.dma_start(tx0[:, 0:2, :], xT_r[0][:, 0:2, :])
            nc.gpsimd.dma_start(ab[:], ab_d[:])
            nc.sync.dma_start(tx0[:, 2:4, :], xT_r[0][:, 2:4, :])
            nc.scalar.dma_start(cs[:], cs_d[:])
            load_win(1)                   # sync: j2-3
            # gpsimd gate: wout/xts1 streams wait until xts0 has landed
            nc.gpsimd.tensor_copy(gate3[:], tx0[0:1, :, 0:1])
            load_wout(0)
            for i in range(2, len(WIN_PARTS)):
                load_win(i)
            for i in range(1, len(WOUT_PARTS)):
                load_wout(i)
            load_xts(1, nc.gpsimd)

            def win_slice(j, k):
                for i, (a, b) in enumerate(WIN_PARTS):
                    if a <= j < b:
                        return win_c[i][:, k, 128 * (j - a) : 128 * (j - a + 1)]
                raise AssertionError

            def wout_slice(j, j2):
                for i, (a, b) in enumerate(WOUT_PARTS):
                    if a <= j < b:
                        return wout_g[i][:, j - a, 128 * j2 : 128 * (j2 + 1)]
                raise AssertionError

            # ---- PE warm-up on scratch (HAM to K=8/8 by ~10.5us) ----
            for w in range(NWARM):
                pw = pap.tile([128, QCH], F32, name=f"warm{w}", tag="pa")
                nc.tensor.matmul(pw[:], wmt[:, 0:128], wmt[:], start=True, stop=True)

            def emit_yo_batch(pq, ppys):
                # 2 copies on Act, 2 on DVE, stores on the idle sync queue
                yos = []
                for j2 in range(NK):
                    yo = yop.tile([128, QCH], BF16, name=f"yo{pq}_{j2}", tag="yo")
                    if j2 % 2 == 0:
                        nc.scalar.copy(yo[:], ppys[j2][:])
                    else:
                        nc.vector.tensor_copy(yo[:], ppys[j2][:])
                    yos.append(yo)
                for j2 in range(NK):
                    nc.sync.dma_start(
                        yT_d[128 * j2 : 128 * (j2 + 1),
                             pq * QCH : (pq + 1) * QCH],
                        yos[j2][:],
                    )

            # ---- fused main loop: j-pairs, mm2 pipelined one pair behind ----
            prev_q = None  # (q, pys) drained at pair 1 of the next q
            for q in range(NQ):
                lo, hi = q * QCH, (q + 1) * QCH
                pys = [pyp.tile([128, QCH], F32, name=f"py{q}_{j2}", tag="py")
                       for j2 in range(NK)]
                pend = None
                for p in range(NJ // 2):
                    j0, j1 = 2 * p, 2 * p + 1
                    pa0 = pap.tile([128, QCH], F32, tag="pa")
                    for k in range(NK):
                        nc.tensor.matmul(pa0[:], win_slice(j0, k),
                                         xts_q[q][:, k, :],
                                         start=(k == 0), stop=(k == NK - 1))
                    pb0 = pbp.tile([128, QCH], F32, tag="pb")
                    nc.tensor.matmul(pb0[:], ab[0:K2, 128 * j0 : 128 * (j0 + 1)],
                                     cs[0:K2, lo:hi], start=True, stop=True)
                    pb1 = pbp.tile([128, QCH], F32, tag="pb")
                    nc.tensor.matmul(pb1[:],
                                     ab[K2 : 2 * K2, 128 * j1 : 128 * (j1 + 1)],
                                     cs[K2 : 2 * K2, lo:hi],
                                     start=True, stop=True)
                    msb0 = modsp.tile([128, QCH], F32, tag="mods")
                    nc.scalar.copy(msb0[:], pb0[:])
                    hm0 = hmp.tile([128, QCH], BF16, tag="hm")
                    nc.vector.tensor_mul(hm0[:], pa0[:], msb0[:])
                    pa1 = pap.tile([128, QCH], F32, tag="pa")
                    for k in range(NK):
                        nc.tensor.matmul(pa1[:], win_slice(j1, k),
                                         xts_q[q][:, k, :],
                                         start=(k == 0), stop=(k == NK - 1))
                    msb1 = modsp.tile([128, QCH], F32, tag="mods")
                    nc.scalar.copy(msb1[:], pb1[:])
                    hm1 = hmp.tile([128, QCH], BF16, tag="hm")
                    nc.vector.tensor_mul(hm1[:], pa1[:], msb1[:])
                    # previous q's outputs drain here (after pair 0's msb/mul
                    # so the pa/pb recycle chain is never behind the copies)
                    if p == 1 and prev_q is not None:
                        pq, ppys = prev_q
                        emit_yo_batch(pq, ppys)
                        prev_q = None
                    if pend is not None:
                        for (pj, phm) in pend:
                            for j2 in range(NK):
                                nc.tensor.matmul(
                                    pys[j2][:],
                                    wout_slice(pj, j2),
                                    phm[:],
                                    start=(pj == 0),
                                    stop=(pj == NJ - 1),
                                )
                    pend = [(j0, hm0), (j1, hm1)]
                # flush last pair's mm2; for the final q the yo copies chase
                # the per-j2 stop matmuls so the tail is one copy deep
                last_q = q == NQ - 1
                yos = []
                for (pj, phm) in pend:
                    for j2 in range(NK):
                        nc.tensor.matmul(
                            pys[j2][:],
                            wout_slice(pj, j2),
                            phm[:],
                            start=(pj == 0),
                            stop=(pj == NJ - 1),
                        )
                        if last_q and pj == NJ - 1:
                            yo = yop.tile([128, QCH], BF16,
                                          name=f"yo{q}_{j2}", tag="yo")
                            if j2 % 2 == 0:
                                nc.scalar.copy(yo[:], pys[j2][:])
                            else:
                                nc.vector.tensor_copy(yo[:], pys[j2][:])
                            yos.append((j2, yo))
                for j2, yo in yos:
                    eng = (nc.sync, nc.sync, nc.scalar, nc.gpsimd)[j2]
                    eng.dma_start(
                        yT_d[128 * j2 : 128 * (j2 + 1), lo:hi], yo[:]
                    )
                prev_q = (q, pys)

    nc.finalize()
    return nc


def _get_nc():
    if "nc" not in _cache:
        _cache["nc"] = _build()
    return _cache["nc"]


def _bf16(a):
    import ml_dtypes
    return np.ascontiguousarray(np.asarray(a, dtype=np.float32).astype(ml_dtypes.bfloat16))


def _in_maps(x, input_proj, output_proj, floquet_energies, drive_weights,
             coupling_matrix):
    coupled = coupling_matrix.astype(np.float64) @ drive_weights.astype(np.float64)
    E = floquet_energies.astype(np.float64)
    ab_half = np.concatenate(
        [coupled[:, None] * np.cos(E), -coupled[:, None] * np.sin(E)], axis=0
    )
    ab = _bf16(np.concatenate([ab_half, ab_half], axis=0))
    w_in = _bf16(input_proj)
    w_out = _bf16(output_proj)

    harm = np.arange(1, M + 1, dtype=np.float64)
    maps = []
    for c in range(NCORES):
        b, half = c // 2, c % 2
        t = (half * T + np.arange(T, dtype=np.float64)) / L
        ang = 2.0 * np.pi * harm[:, None] * t[None, :]
        cs_half = np.concatenate([np.cos(ang), np.sin(ang)], axis=0)
        cs = _bf16(np.concatenate([cs_half, cs_half], axis=0))
        xT = _bf16(x[b, half * T : (half + 1) * T, :].T)
        maps.append(
            {
                "xT": xT,
                "w_in": w_in,
                "w_out": w_out,
                "cs": cs,
                "ab": ab,
            }
        )
    return maps


def kernel(x, input_proj, output_proj, floquet_energies, drive_weights,
           coupling_matrix, _trace=False, _trace_kwargs=None):
    from concourse.bass_utils import run_bass_kernel_spmd

    nc = _get_nc()
    maps = _in_maps(x, input_proj, output_proj, floquet_energies,
                    drive_weights, coupling_matrix)
    kw = dict(_trace_kwargs or {})
    res = run_bass_kernel_spmd(nc, maps, list(range(NCORES)), trace=_trace, **kw)
    out = np.empty((B, L, D), dtype=np.float32)
    for c in range(NCORES):
        b, half = c // 2, c % 2
        out[b, half * T : (half + 1) * T, :] = (
            res.results[c]["yT"].astype(np.float32).T
        )
    if _trace:
        return out, res
    return out


# revision 24
# speedup vs baseline: 1.0235x; 1.0209x over previous
"""HDTimeCrystalBlock kernel for 8 Trainium2 NeuronCores.

Math: out = ((x @ W_in) * mod[None]) @ W_out, where
  mod[l,h] = sum_m coupled[m] * cos(omega*(m+1)*t[l] + E[m,h])
With cos(a+b) = cos(a)cos(b) - sin(a)sin(b), mod is a K=32 matmul:
  mod[h,l] = sum_r ab[r,h] * cs[r,l]
  ab rows 0:16  = coupled[m]*cos(E[m,h]),  rows 16:32 = -coupled[m]*sin(E[m,h])
  cs rows 0:16  = cos(omega*(m+1)*t[l]),   rows 16:32 = sin(...)
ab/cs are tiny and precomputed on the HOST (the old on-device prep chain
kept the PE HAM-throttled at 1.2 GHz for the first ~35us).  Both are
shipped with the 32 rows duplicated at partitions 32:64 so consecutive
j-tiles' mod matmuls run CONCURRENTLY in different 32-row strips of the
PE array (row tiling): a pair of K=32 matmuls costs ~one N=512 slot.

Sharding: data-parallel over the 8192 tokens (B*L), 1024 per core; weights
replicated. Activations stay transposed ([feature, token]); all matmul
operands bf16 (1 col/cycle @ 2.4 GHz). Output stored bf16.

Startup discipline:
  - PE warm-up matmuls on a memset scratch tile (no DMA dependency): PE
    busy from ~7us, HAM un-throttles by ~10.5us when real data lands.
  - DMA queue heads are exactly the first-needed tiles (win j0-1 on sync,
    xts0 on scalar, ab+cs on gpsimd); the remaining ~8.5MB of weight
    streams are gated behind a 1-element copy that depends on xts0, so
    they cannot steal HBM bandwidth from the critical path.
Steady-loop discipline:
  - The main loop runs in j-PAIRS; the mm2 stage is software-pipelined one
    pair (two j) behind, so the pb->msb->hm chain has ~4.5us of slack.
  - mm2 writes 8 half-bank PSUM tiles ([128,256] x 2 halves x 4 d-tiles =
    4 banks, pool bufs=8) so consecutive q's outputs double-buffer and the
    yo output copies (interleaved on Act/DVE during the next q) are never
    on the critical path.
"""
import math

import numpy as np

B, L, D, HD, M = 4, 2048, 512, 4096, 16
NCORES = 8
T = (B * L) // NCORES          # tokens per core
QCH = 512                      # l-chunk (PSUM bank width in fp32)
HCH = QCH // 2                 # mm2 half-tile width
NQ = T // QCH
NJ = HD // 128                 # h-tiles
NK = D // 128                  # d-tiles
K2 = 2 * M                     # mod-matmul contraction (32)
NWARM = 10                     # scratch warm-up matmuls (N=512)

# j-tile ranges per DMA chunk for w_in / w_out ([lo, hi) in j-tiles)
WIN_PARTS = [(0, 2), (2, 4), (4, 8), (8, 12), (12, 16), (16, 20),
             (20, 24), (24, 28), (28, 32)]
WOUT_PARTS = [(0, 4), (4, 8), (8, 12), (12, 16), (16, 20),
              (20, 24), (24, 28), (28, 32)]

_cache = {}


def _build():
    from concourse import bacc, bass, mybir, tile

    F32 = mybir.dt.float32
    BF16 = mybir.dt.bfloat16
    PSUM = bass.MemorySpace.PSUM

    nc = bacc.Bacc("TRN2", target_bir_lowering=False, debug=False)

    xT_d = nc.dram_tensor("xT", [D, T], BF16, kind="ExternalInput")
    w_in_d = nc.dram_tensor("w_in", [D, HD], BF16, kind="ExternalInput")
    w_out_d = nc.dram_tensor("w_out", [HD, D], BF16, kind="ExternalInput")
    cs_d = nc.dram_tensor("cs", [2 * K2, T], BF16, kind="ExternalInput")
    ab_d = nc.dram_tensor("ab", [2 * K2, HD], BF16, kind="ExternalInput")
    yT_d = nc.dram_tensor("yT", [D, T], BF16, kind="ExternalOutput")

    with tile.TileContext(nc) as tc:
        with (
            tc.tile_pool(name="win", bufs=1) as winp,
            tc.tile_pool(name="wout", bufs=1) as woutp,
            tc.tile_pool(name="xts", bufs=1) as xtp,
            tc.tile_pool(name="small", bufs=1) as smallp,
            tc.tile_pool(name="hm", bufs=4) as hmp,
            tc.tile_pool(name="mods", bufs=4) as modsp,
            tc.tile_pool(name="yo", bufs=4) as yop,
            tc.tile_pool(name="pa", bufs=2, space=PSUM) as pap,
            tc.tile_pool(name="pb", bufs=2, space=PSUM) as pbp,
            tc.tile_pool(name="py", bufs=4, space=PSUM) as pyp,
        ):
            # ---- scratch warm-up tile, memset on the otherwise-idle DVE ----
            wmt = smallp.tile([128, QCH], BF16, tag="wmt")
            nc.vector.memset(wmt[:], 0.5)

            # ---- DMA schedule ----
            w_in_r = w_in_d.ap().rearrange("(k p) h -> p k h", p=128)
            w_out_r = w_out_d.ap().rearrange("(j p) i -> p j i", p=128)
            xT_r = xT_d.ap().rearrange("(k p) (q t) -> q p k t", p=128, q=NQ)

            win_c = [None] * len(WIN_PARTS)
            wout_g = [None] * len(WOUT_PARTS)
            xts_q = [None] * NQ

            def load_win(i):
                a, b = WIN_PARTS[i]
                t_ = winp.tile([128, NK, 128 * (b - a)], BF16,
                               name=f"win{i}", tag=f"win{i}")
                nc.sync.dma_start(t_[:], w_in_r[:, :, 128 * a : 128 * b])
                win_c[i] = t_

            def load_wout(i):
                a, b = WOUT_PARTS[i]
                tw = woutp.tile([128, b - a, D], BF16,
                                name=f"wout{i}", tag=f"wout{i}")
                nc.gpsimd.dma_start(tw[:], w_out_r[:, a:b, :])
                wout_g[i] = tw

            def load_xts(q, eng):
                tx = xtp.tile([128, NK, T // NQ], BF16, name=f"xts{q}", tag=f"xts{q}")
                eng.dma_start(tx[:], xT_r[q])
                xts_q[q] = tx

            ab = smallp.tile([2 * K2, HD], BF16, tag="ab")
            cs = smallp.tile([2 * K2, T], BF16, tag="cs")

            # DMA engines serve ALL outstanding transfers byte-fair
            # round-robin, so priority = don't let later chunks be in
            # flight.  The critical set (win j0-1, xts0, ab, cs) is issued
            # alone; every later chunk is chained behind an earlier tile
            # via a 1-element gpsimd copy into its destination (WAW forces
            # the DMA to wait), forming two streams in consumption order.
            tx0 = xtp.tile([128, NK, T // NQ], BF16, name="xts0", tag="xts0")
            xts_q[0] = tx0
            load_win(0)                   # sync: j0-1
            nc.scalar.dma_start(tx0[:, 0:2, :], xT_r[0][:, 0:2, :])
            nc.gpsimd.dma_start(ab[:], ab_d[:])
            nc.sync.dma_start(tx0[:, 2:4, :], xT_r[0][:, 2:4, :])
            nc.scalar.dma_start(cs[:], cs_d[:])

            def chain(dst_tile, src_tile, tag):
                # 1-elem pre-write of dst from src: dst's DMA must follow
                # src's landing; the real DMA then overwrites it.
                # (all chained tiles here are 3D)
                nc.gpsimd.tensor_copy(dst_tile[0:1, 0:1, 0:1],
                                      src_tile[0:1, 0:1, 0:1])

            def win_tile(i):
                a, b = WIN_PARTS[i]
                t_ = winp.tile([128, NK, 128 * (b - a)], BF16,
                               name=f"win{i}", tag=f"win{i}")
                win_c[i] = t_
                return t_

            def wout_tile(i):
                a, b = WOUT_PARTS[i]
                tw = woutp.tile([128, b - a, D], BF16,
                                name=f"wout{i}", tag=f"wout{i}")
                wout_g[i] = tw
                return tw

            def win_dma(i):
                a, b = WIN_PARTS[i]
                nc.sync.dma_start(win_c[i][:], w_in_r[:, :, 128 * a : 128 * b])

            def wout_dma(i):
                a, b = WOUT_PARTS[i]
                nc.gpsimd.dma_start(wout_g[i][:], w_out_r[:, a:b, :])

            t_ = win_tile(1)
            chain(t_, tx0, "w1")
            win_dma(1)
            tw = wout_tile(0)
            chain(tw, tx0, "o0")
            wout_dma(0)
            t_ = win_tile(2)
            chain(t_, win_c[1], "w2")
            win_dma(2)
            tw = wout_tile(1)
            chain(tw, wout_g[0], "o1")
            wout_dma(1)
            tx1 = xtp.tile([128, NK, T // NQ], BF16, name="xts1", tag="xts1")
            xts_q[1] = tx1
            chain(tx1, wout_g[0], "x1")
            nc.gpsimd.dma_start(tx1[:], xT_r[1])
            for i in range(3, len(WIN_PARTS)):
                t_ = win_tile(i)
                chain(t_, win_c[i - 1], f"w{i}")
                win_dma(i)
                tw = wout_tile(i - 1)
                chain(tw, wout_g[i - 2], f"o{i-1}")
                wout_dma(i - 1)
            tw = wout_tile(7)
            chain(tw, wout_g[6], "o7")
            wout_dma(7)

            def win_slice(j, k):
                for i, (a, b) in enumerate(WIN_PARTS):
                    if a <= j < b:
                        return win_c[i][:, k, 128 * (j - a) : 128 * (j - a + 1)]
                raise AssertionError

            def wout_slice(j, j2):
                for i, (a, b) in enumerate(WOUT_PARTS):
                    if a <= j < b:
                        return wout_g[i][:, j - a, 128 * j2 : 128 * (j2 + 1)]
                raise AssertionError

            # ---- PE warm-up on scratch (HAM to K=8/8 by ~10.5us) ----
            for w in range(NWARM):
                pw = pap.tile([128, QCH], F32, name=f"warm{w}", tag="pa")
                nc.tensor.matmul(pw[:], wmt[:, 0:128], wmt[:], start=True, stop=True)

            def emit_yo_batch(pq, ppys):
                # 2 copies on Act, 2 on DVE, stores on the idle sync queue
                yos = []
                for j2 in range(NK):
                    yo = yop.tile([128, QCH], BF16, name=f"yo{pq}_{j2}", tag="yo")
                    if j2 % 2 == 0:
                        nc.scalar.copy(yo[:], ppys[j2][:])
                    else:
                        nc.vector.tensor_copy(yo[:], ppys[j2][:])
                    yos.append(yo)
                for j2 in range(NK):
                    nc.sync.dma_start(
                        yT_d[128 * j2 : 128 * (j2 + 1),
                             pq * QCH : (pq + 1) * QCH],
                        yos[j2][:],
                    )

            # ---- fused main loop: j-pairs, mm2 pipelined one pair behind ----
            prev_q = None  # (q, pys) drained at pair 1 of the next q
            for q in range(NQ):
                lo, hi = q * QCH, (q + 1) * QCH
                pys = [pyp.tile([128, QCH], F32, name=f"py{q}_{j2}", tag="py")
                       for j2 in range(NK)]
                pend = None
                for p in range(NJ // 2):
                    j0, j1 = 2 * p, 2 * p + 1
                    pa0 = pap.tile([128, QCH], F32, tag="pa")
                    for k in range(NK):
                        nc.tensor.matmul(pa0[:], win_slice(j0, k),
                                         xts_q[q][:, k, :],
                                         start=(k == 0), stop=(k == NK - 1))
                    pb0 = pbp.tile([128, QCH], F32, tag="pb")
                    nc.tensor.matmul(pb0[:], ab[0:K2, 128 * j0 : 128 * (j0 + 1)],
                                     cs[0:K2, lo:hi], start=True, stop=True)
                    pb1 = pbp.tile([128, QCH], F32, tag="pb")
                    nc.tensor.matmul(pb1[:],
                                     ab[K2 : 2 * K2, 128 * j1 : 128 * (j1 + 1)],
                                     cs[K2 : 2 * K2, lo:hi],
                                     start=True, stop=True)
                    msb0 = modsp.tile([128, QCH], F32, tag="mods")
                    nc.scalar.copy(msb0[:], pb0[:])
                    hm0 = hmp.tile([128, QCH], BF16, tag="hm")
                    nc.vector.tensor_mul(hm0[:], pa0[:], msb0[:])
                    pa1 = pap.tile([128, QCH], F32, tag="pa")
                    for k in range(NK):
                        nc.tensor.matmul(pa1[:], win_slice(j1, k),
                                         xts_q[q][:, k, :],
                                         start=(k == 0), stop=(k == NK - 1))
                    msb1 = modsp.tile([128, QCH], F32, tag="mods")
                    nc.scalar.copy(msb1[:], pb1[:])
                    hm1 = hmp.tile([128, QCH], BF16, tag="hm")
                    nc.vector.tensor_mul(hm1[:], pa1[:], msb1[:])
                    # previous q's outputs drain here (after pair 0's msb/mul
                    # so the pa/pb recycle chain is never behind the copies)
                    if p == 1 and prev_q is not None:
                        pq, ppys = prev_q
                        emit_yo_batch(pq, ppys)
                        prev_q = None
                    if pend is not None:
                        for (pj, phm) in pend:
                            for j2 in range(NK):
                                nc.tensor.matmul(
                                    pys[j2][:],
                                    wout_slice(pj, j2),
                                    phm[:],
                                    start=(pj == 0),
                                    stop=(pj == NJ - 1),
                                )
                    pend = [(j0, hm0), (j1, hm1)]
                # flush last pair's mm2; for the final q the yo copies chase
                # the per-j2 stop matmuls so the tail is one copy deep
                last_q = q == NQ - 1
                yos = []
                for (pj, phm) in pend:
                    for j2 in range(NK):
                        nc.tensor.matmul(
                            pys[j2][:],
                            wout_slice(pj, j2),
                            phm[:],
                            start=(pj == 0),
                            stop=(pj == NJ - 1),
                        )
                        if last_q and pj == NJ - 1:
                            yo = yop.tile([128, QCH], BF16,
                                          name=f"yo{q}_{j2}", tag="yo")
                            if j2 % 2 == 0:
                                nc.scalar.copy(yo[:], pys[j2][:])
                            else:
                                nc.vector.tensor_copy(yo[:], pys[j2][:])
                            yos.append((j2, yo))
                for j2, yo in yos:
                    eng = (nc.sync, nc.sync, nc.scalar, nc.gpsimd)[j2]
                    eng.dma_start(
                        yT_d[128 * j2 : 128 * (j2 + 1), lo:hi], yo[:]
                    )
                prev_q = (q, pys)

    nc.finalize()
    return nc


def _get_nc():
    if "nc" not in _cache:
        _cache["nc"] = _build()
    return _cache["nc"]


def _bf16(a):
    import ml_dtypes
    return np.ascontiguousarray(np.asarray(a, dtype=np.float32).astype(ml_dtypes.bfloat16))


def _in_maps(x, input_proj, output_proj, floquet_energies, drive_weights,
             coupling_matrix):
    coupled = coupling_matrix.astype(np.float64) @ drive_weights.astype(np.float64)
    E = floquet_energies.astype(np.float64)
    ab_half = np.concatenate(
        [coupled[:, None] * np.cos(E), -coupled[:, None] * np.sin(E)], axis=0
    )
    ab = _bf16(np.concatenate([ab_half, ab_half], axis=0))
    w_in = _bf16(input_proj)
    w_out = _bf16(output_proj)

    harm = np.arange(1, M + 1, dtype=np.float64)
    maps = []
    for c in range(NCORES):
        b, half = c // 2, c % 2
        t = (half * T + np.arange(T, dtype=np.float64)) / L
        ang = 2.0 * np.pi * harm[:, None] * t[None, :]
        cs_half = np.concatenate([np.cos(ang), np.sin(ang)], axis=0)
        cs = _bf16(np.concatenate([cs_half, cs_half], axis=0))
        xT = _bf16(x[b, half * T : (half + 1) * T, :].T)
        maps.append(
            {
                "xT": xT,
                "w_in": w_in,
                "w_out": w_out,
                "cs": cs,
                "ab": ab,
            }
        )
    return maps


def kernel(x, input_proj, output_proj, floquet_energies, drive_weights,
           coupling_matrix, _trace=False, _trace_kwargs=None):
    from concourse.bass_utils import run_bass_kernel_spmd

    nc = _get_nc()
    maps = _in_maps(x, input_proj, output_proj, floquet_energies,
                    drive_weights, coupling_matrix)
    kw = dict(_trace_kwargs or {})
    res = run_bass_kernel_spmd(nc, maps, list(range(NCORES)), trace=_trace, **kw)
    out = np.empty((B, L, D), dtype=np.float32)
    for c in range(NCORES):
        b, half = c // 2, c % 2
        out[b, half * T : (half + 1) * T, :] = (
            res.results[c]["yT"].astype(np.float32).T
        )
    if _trace:
        return out, res
    return out
